# revision 6
# baseline (speedup 1.0000x reference)
"""Trainium2 Bass kernel for AbstractRelationalGraphConvolution.

Math (per reference):
    self_out = E[nodes] @ W.T                                   [B, D]
    nmask    = masks / (masks.sum(-1, keepdims=True) + eps)     [R, B, U]
    agg      = einsum('rbu,ud->rbd', nmask, E[uidx])            [R, B, D]
    rel_out  = einsum('rbd,rod->bo', agg, rel_weights)          [B, D]
    out      = relu(self_out + rel_out)

Sharding: batch B across 8 cores (data parallel, no collectives).

Per-core algorithm (memory-bound on the 64 MiB mask slice):
  * masks are 0.0/1.0 f32; the high 16 bits of each f32 are its exact bf16.
    We DMA-xbar-transpose the mask slice DIRECTLY from DRAM, reinterpreted as
    bf16 pairs: output tile MT[p, c, b] = bf16view[b, 128c + p].  Odd
    partitions hold bf16 mask values for u = 64c + (p-1)/2, even partitions
    hold the (zero) low halves.  Zero rows are harmless in the matmul
    contraction, so each K=128 matmul contracts 64 real u values.
  * The neighbor table is gathered with DUPLICATED indices (u at partitions
    2j and 2j+1) so it pairs with the interleaved transposed masks, plus an
    appended ones-column which makes the PE accumulate the mask row-sums for
    free (psum column D).
  * agg_raw rows are scaled by 1/(rowsum+eps) during the PSUM->SBUF copy
    (ACT activation scale), transposed on the PE, and contracted with the
    pre-transposed relation weights; the self path accumulates into the same
    PSUM tile; ReLU on the way out.
"""

import numpy as np

from concourse import bass, mybir, tile, bacc
from concourse.bass import IndirectOffsetOnAxis
from concourse.bass_utils import run_bass_kernel_spmd
from concourse.masks import make_identity

F32 = mybir.dt.float32
BF16 = mybir.dt.bfloat16
I32 = mybir.dt.int32
AF = mybir.ActivationFunctionType

N_CORES = 8
N_NODES = 100000
D = 256
B = 2048
U = 8192
R = 8
EPS = 1e-10


def build_graphconv(
    n_nodes=N_NODES,
    d=D,
    b_loc=B // N_CORES,
    u=U,
    r=R,
    eps=EPS,
    gather_group=16,
    repeat=1,
):
    """Build the single-core program (run SPMD on 8 cores).

    repeat > 1 re-runs the main loop (for steady-state timing); the
    computation is idempotent so results are unchanged.
    """
    assert d == 256 and b_loc % 128 == 0 and u % 64 == 0
    BC = b_loc // 128  # batch chunks of 128
    UC = u // 64  # u chunks of 64 (each -> 128 interleaved partitions)
    DC = d // 128  # always 2

    nc = bacc.Bacc("TRN2", target_bir_lowering=False, debug=False, num_devices=1)

    nodes_t = nc.dram_tensor("nodes", [b_loc], I32, kind="ExternalInput")
    uidx_t = nc.dram_tensor("unique_idx", [u], I32, kind="ExternalInput")
    masks_t = nc.dram_tensor("masks", [r, b_loc, u], F32, kind="ExternalInput")
    emb_t = nc.dram_tensor("embeddings", [n_nodes, d], F32, kind="ExternalInput")
    w_t = nc.dram_tensor("weight", [d, d], F32, kind="ExternalInput")
    rw_t = nc.dram_tensor("rel_weights", [r, d, d], F32, kind="ExternalInput")
    out_t = nc.dram_tensor("out", [b_loc, d], F32, kind="ExternalOutput")

    with tile.TileContext(nc) as tc:
        with (
            tc.tile_pool(name="const", bufs=1) as constp,
            tc.tile_pool(name="mt", bufs=2) as mtp,
            tc.tile_pool(name="small", bufs=2) as smallp,
            tc.tile_pool(name="psA", bufs=2, space="PSUM") as psA,
            tc.tile_pool(name="psB", bufs=2, space="PSUM") as psB,
            tc.tile_pool(name="psO", bufs=1, space="PSUM") as psO,
        ):
            # ---- constants -------------------------------------------------
            ident = constp.tile([128, 128], F32)
            make_identity(nc, ident[:])
            ident_bf = constp.tile([128, 128], BF16)
            nc.scalar.copy(ident_bf[:], ident[:])

            # ---- weights: load (cast bf16) and pre-transpose ---------------
            # W_sb[p, oc, :] = W[oc*128+p, :]
            w_sb = constp.tile([128, DC, d], BF16)
            nc.gpsimd.dma_start(
                out=w_sb[:], in_=w_t.ap().rearrange("(oc p) d -> p oc d", p=128)
            )
            # WT[p, dc, o] = W[o, dc*128+p]
            wT = constp.tile([128, DC, d], BF16)
            for oc in range(DC):
                for dc in range(DC):
                    tp = psA.tile([128, 128], BF16, tag="tp")
                    nc.tensor.transpose(
                        tp[:], w_sb[:, oc, dc * 128 : (dc + 1) * 128], ident_bf[:]
                    )
                    nc.scalar.copy(wT[:, dc, oc * 128 : (oc + 1) * 128], tp[:])

            rw_sb = constp.tile([128, r, DC, d], BF16)
            for ri in range(r):
                nc.gpsimd.dma_start(
                    out=rw_sb[:, ri, :, :],
                    in_=rw_t.ap()[ri].rearrange("(oc p) d -> p oc d", p=128),
                )
            # WrT[p, dc, ri, o] = Wr[ri, o, dc*128+p]
            wrT = constp.tile([128, DC, r, d], BF16)
            for ri in range(r):
                for oc in range(DC):
                    for dc in range(DC):
                        tp = psA.tile([128, 128], BF16, tag="tp")
                        nc.tensor.transpose(
                            tp[:],
                            rw_sb[:, ri, oc, dc * 128 : (dc + 1) * 128],
                            ident_bf[:],
                        )
                        nc.scalar.copy(
                            wrT[:, dc, ri, oc * 128 : (oc + 1) * 128], tp[:]
                        )

            # ---- neighbor index layout -------------------------------------
            # L[c, j] = uidx[64c + j]  (c on partitions, c in [0, UC))
            assert UC <= 128
            idxL = constp.tile([UC, 64], I32)
            nc.sync.dma_start(
                out=idxL[:], in_=uidx_t.ap().rearrange("(c j) -> c j", j=64)
            )
            idxLf = constp.tile([UC, 64], F32)
            nc.vector.tensor_copy(idxLf[:], idxL[:])
            # Gather-index tile: G[p, c] = uidx[64c + p//2].
            # Duplicate each j into adjacent columns, then transpose:
            # in_[c, m] = Lf[c, m//2] -> out[m, c] = uidx[64c + m//2].
            gidx = constp.tile([128, UC], I32)
            gidx_f = constp.tile([128, UC], F32, tag="gidx_f")
            idxLdup = constp.tile([UC, 128], F32)
            nc.vector.tensor_copy(
                idxLdup[:].rearrange("c (j t) -> c j t", t=2),
                idxLf[:].unsqueeze(2).broadcast_to([UC, 64, 2]),
            )
            tpg = psA.tile([128, 128], F32, tag="tpf")
            nc.tensor.transpose(tpg[:, :UC], idxLdup[:], ident[:UC, :UC])
            nc.vector.tensor_copy(gidx_f[:], tpg[:, :UC])
            nc.vector.tensor_copy(gidx[:], gidx_f[:])

            # ---- self-path index layout ------------------------------------
            # A[q, i] = nodes[128q + i]; transpose -> idx2[p, c] = nodes[128c+p]
            idxA = constp.tile([BC, 128], I32)
            nc.sync.dma_start(
                out=idxA[:], in_=nodes_t.ap().rearrange("(q i) -> q i", i=128)
            )
            idxAf = constp.tile([BC, 128], F32)
            nc.vector.tensor_copy(idxAf[:], idxA[:])
            tp2 = psA.tile([128, 128], F32, tag="tpf")
            nc.tensor.transpose(tp2[:, :BC], idxAf[:], ident[:BC, :BC])
            idx2f = constp.tile([128, BC], F32)
            nc.vector.tensor_copy(idx2f[:], tp2[:, :BC])
            idx2 = constp.tile([128, BC], I32)
            nc.vector.tensor_copy(idx2[:], idx2f[:])

            # ---- gathers ---------------------------------------------------
            # NEI[p, c, 0:256] = E[uidx[64c + p//2]] (bf16), NEI[p, c, 256] = 1
            nei = constp.tile([128, UC, d + 1], BF16)
            nc.vector.memset(nei[:, :, d : d + 1], 1.0)
            for g0 in range(0, UC, gather_group):
                gw = min(gather_group, UC - g0)
                nc.gpsimd.indirect_dma_start(
                    out=nei[:, g0 : g0 + gw, 0:d],
                    out_offset=None,
                    in_=emb_t.ap(),
                    in_offset=IndirectOffsetOnAxis(ap=gidx[:, g0 : g0 + gw], axis=0),
                )
            # EN[p, c, :] = E[nodes[128c + p]] (bf16)
            en = constp.tile([128, BC, d], BF16)
            nc.gpsimd.indirect_dma_start(
                out=en[:],
                out_offset=None,
                in_=emb_t.ap(),
                in_offset=IndirectOffsetOnAxis(ap=idx2[:], axis=0),
            )
            # ENT[p, dc, bc, j] = E[nodes[128bc + j], dc*128 + p]
            ent = constp.tile([128, DC, BC, 128], BF16)
            for bc in range(BC):
                for dc in range(DC):
                    tp = psA.tile([128, 128], BF16, tag="tp")
                    nc.tensor.transpose(
                        tp[:], en[:, bc, dc * 128 : (dc + 1) * 128], ident_bf[:]
                    )
                    nc.scalar.copy(ent[:, dc, bc, :], tp[:])

            # ---- main loop -------------------------------------------------
            for rep in range(repeat):
              for bc in range(BC):
                out_ps = psO.tile([128, d], F32, tag="out_ps")
                # self path: out_ps[b, o] = sum_d E[nodes[b], d] * W[o, d]
                for dc in range(DC):
                    nc.tensor.matmul(
                        out_ps[:],
                        lhsT=ent[:, dc, bc, :],
                        rhs=wT[:, dc, :],
                        start=(dc == 0),
                        stop=False,
                        skip_group_check=True,
                    )
                for ri in range(r):
                    mt = mtp.tile([128, UC, 128], BF16, tag="mt")
                    nc.sync.dma_start(
                        out=mt[:],
                        in_=masks_t.ap()[ri, bc * 128 : (bc + 1) * 128, :].bitcast(
                            BF16
                        ),
                        transpose=True,
                    )
                    agg_ps = psB.tile([128, d + 1], F32, tag="agg")
                    for c in range(UC):
                        nc.tensor.matmul(
                            agg_ps[:],
                            lhsT=mt[:, c, :],
                            rhs=nei[:, c, :],
                            start=(c == 0),
                            stop=(c == UC - 1),
                        )
                    # recip = 1 / (rowsum + eps)
                    rs = smallp.tile([128, 1], F32, tag="rs")
                    nc.vector.tensor_scalar_add(rs[:], agg_ps[:, d : d + 1], eps)
                    recip = smallp.tile([128, 1], F32, tag="recip")
                    nc.vector.reciprocal(recip[:], rs[:])
                    # normalize + cast during PSUM->SBUF copy
                    agg_sb = smallp.tile([128, d], BF16, tag="agg_sb")
                    nc.scalar.mul(agg_sb[:], agg_ps[:, 0:d], recip[:, 0:1])
                    # transpose agg and contract with rel weights
                    for dc in range(DC):
                        tp = psA.tile([128, 128], BF16, tag="tp")
                        nc.tensor.transpose(
                            tp[:], agg_sb[:, dc * 128 : (dc + 1) * 128], ident_bf[:]
                        )
                        aggT = smallp.tile([128, 128], BF16, tag="aggT")
                        nc.scalar.copy(aggT[:], tp[:])
                        nc.tensor.matmul(
                            out_ps[:],
                            lhsT=aggT[:],
                            rhs=wrT[:, dc, ri, :],
                            start=False,
                            stop=(ri == r - 1 and dc == DC - 1),
                            skip_group_check=True,
                        )
                out_sb = smallp.tile([128, d], F32, tag="out_sb")
                nc.scalar.activation(out_sb[:], out_ps[:], AF.Relu)
                nc.sync.dma_start(
                    out=out_t.ap()[bc * 128 : (bc + 1) * 128, :], in_=out_sb[:]
                )

    nc.compile()
    return nc


_CACHED_NC = None


def get_nc():
    global _CACHED_NC
    if _CACHED_NC is None:
        _CACHED_NC = build_graphconv()
    return _CACHED_NC


def make_in_maps(nodes, unique_idx, masks, embeddings, weight, rel_weights):
    nodes = np.ascontiguousarray(np.asarray(nodes).astype(np.int32))
    unique_idx = np.ascontiguousarray(np.asarray(unique_idx).astype(np.int32))
    masks = np.ascontiguousarray(np.asarray(masks, dtype=np.float32))
    embeddings = np.ascontiguousarray(np.asarray(embeddings, dtype=np.float32))
    weight = np.ascontiguousarray(np.asarray(weight, dtype=np.float32))
    rel_weights = np.ascontiguousarray(np.asarray(rel_weights, dtype=np.float32))

    b_loc = B // N_CORES
    in_maps = []
    for i in range(N_CORES):
        sl = slice(i * b_loc, (i + 1) * b_loc)
        in_maps.append(
            {
                "nodes": nodes[sl],
                "unique_idx": unique_idx,
                "masks": np.ascontiguousarray(masks[:, sl, :]),
                "embeddings": embeddings,
                "weight": weight,
                "rel_weights": rel_weights,
            }
        )
    return in_maps


def kernel(nodes, unique_idx, masks, embeddings, weight, rel_weights) -> np.ndarray:
    nc = get_nc()
    in_maps = make_in_maps(nodes, unique_idx, masks, embeddings, weight, rel_weights)
    res = run_bass_kernel_spmd(nc, in_maps, core_ids=list(range(N_CORES)))
    return np.concatenate([res.results[i]["out"] for i in range(N_CORES)], axis=0)
